# revision 26
# baseline (speedup 1.0000x reference)
"""DeepFM forward on 8 Trainium2 NeuronCores.

Data-parallel: batch 8192 -> 1024 samples/core; tables replicated.

Math (weight-only preprocessing on host):
  logit_b = fm_b + wide_b + deep_b + b_ffn
  A = sym(w2)/2 = V diag(lam) V^T (float64 eigh); E_b = emb[x_b] [TS, F]
  P_b = V^T [E_b*32 | H_b*2048]  (fp8 rhs, one matmul per 4 samples)
  fm_b   = sum_k lam_k/32^2 sum_f P_b[k, f<64]^2
  wide_b + deep_b = <[V^T W3/32 | V^T W1/2048], P_b>_F   (DVE mult+reduce)

Gather strategy: the problem is a per-core random gather of 131072
(sample, slot) rows x 96 B. SWDGE desc-gen runs ~7.3 ns/desc per queue
with 4 queues concurrent, so the kernel is descriptor-generation bound:
  stage 1: per chunk of 256 samples, gather the ~5.7k unique rows per
    25000-row vocab window (4 windows -> 4 balanced queues, 96 B elems)
    into SBUF staging, then HWDGE-writeback into a compact
    [24576 row, 256 B-stride] HBM table per chunk.
  stage 2: dma_gather from the compact table (ids < 24576 fit int16)
    in 4x 8192-idx instructions per chunk, landing
    sel[t-partition, sample, 96B] directly for the matmul.
Software pipeline: all index tiles are prefetched at t=0; stage-1 of
chunk c+1 is emitted BEFORE stage-2 of chunk c so the in-order Pool
sequencer keeps all 4 queues generating continuously.
  dma_gather needs single_packet=False (>=64 descs/lane crashes the
  device otherwise); 96 B elems at 256 B stride need raw
  InstDMAGatherAnt construction (bass's %256 elem assert is
  transpose-only in ucode).
"""

import os
import numpy as np

import concourse.bass as bass
import concourse.mybir as mybir
from concourse import bacc, ap_utils
from concourse.tile import TileContext
from concourse.bass_utils import run_bass_kernel_spmd

BS, TS, VOCAB, F = 8192, 100, 100000, 64
K = 32
NCORES = 8
SPC = BS // NCORES        # 1024 samples per core
EMB_SCALE = 32.0
H_SCALE = 2048.0

WINW = 25000              # vocab rows per window (balanced across queues)
NBANK = 4
BROWS = 32768             # big-table rows per window (aligned region)
ROWB = 256                # big/compact table row stride (bytes)
PAYB = 96                 # used bytes per row: 64 fp8 emb + 32 fp8 h

CHS = 256                 # samples per chunk
NCH = SPC // CHS          # 4 chunks
M1 = 5888                 # stage-1 list length per window per chunk
CRANK = M1 // 128         # staging ranks per window
CROWS = NBANK * M1        # 24576 compact rows per chunk
CB = [k * M1 for k in range(NBANK)]
M1TOT16 = CROWS // 16     # i1 columns per chunk
SUB = 64                  # stage-2 samples per gather (8192 idxs)
NSUB = CHS // SUB         # 4 sub-gathers per chunk
GRP = 4                   # samples per matmul (384 PSUM cols, 1 bank)
SUPER = 8                 # samples per PSUM super-tile (2 matmul groups)

U8 = mybir.dt.uint8
FP8 = mybir.dt.float8e4
BF16 = mybir.dt.bfloat16
F32 = mybir.dt.float32
I16 = mybir.dt.int16

_cached = {}


def _raw_gather(g, out_ap, in_ap, idxs_ap, num_idxs, elem_size, queue_num):
    """dma_gather minus the elem_size%256 assert (non-transpose, HBM src)."""
    assert idxs_ap.dtype == I16
    elem_step = in_ap.ap[0][0]
    stride_bytes = elem_step * mybir.dt.size(in_ap.dtype)
    assert stride_bytes % 256 == 0
    assert ap_utils.ap_is_contiguous(in_ap.ap[1:])
    assert ap_utils.ap_is_contiguous(out_ap.ap[1:])
    assert ap_utils.ap_is_contiguous(idxs_ap.ap[1:])
    assert in_ap.ap[-1][1] == out_ap.ap[-1][1] == elem_size
    _in = g.lower_ap_dma(in_ap, for_custom_bir_dma=True)
    return g.add_instruction(
        mybir.InstDMAGatherAnt(
            name=g.bass.get_next_instruction_name(),
            ins=[*_in, g.lower_ap(idxs_ap),
                 g.lower_val_access(g.to_reg(num_idxs))],
            outs=[g.lower_ap(out_ap)],
            transpose=False, num_idxs=num_idxs, elem_size=elem_size,
            stride_bytes_256=stride_bytes // 256, gen_mode=0,
            single_packet=False, queue_num=queue_num,
            sbuf_tokens_per_rank=0, sbuf_free_dim_per_rank=0,
            sbuf_free_dim_pad_per_rank=0, sbuf_byte_offset=0,
        ))


def build_nc():
    nc = bacc.Bacc("TRN2", target_bir_lowering=False, debug=False,
                   num_devices=NCORES, num_swdge_queues=NBANK)
    btab = nc.dram_tensor("btab", [NBANK * BROWS, ROWB], U8,
                          kind="ExternalInput")
    i1 = nc.dram_tensor("i1", [NCH, 128, M1TOT16], I16,
                        kind="ExternalInput")
    i2 = nc.dram_tensor("i2", [NCH, NSUB, 128, SUB * 128 // 16], I16,
                        kind="ExternalInput")
    vmat = nc.dram_tensor("vmat", [128, TS], BF16, kind="ExternalInput")
    lam = nc.dram_tensor("lam", [TS, 1], F32, kind="ExternalInput")
    onesv = nc.dram_tensor("onesv", [128, 1], F32, kind="ExternalInput")
    linp = nc.dram_tensor("linp", [128, PAYB], BF16, kind="ExternalInput")
    bffn = nc.dram_tensor("bffn", [1, 1], F32, kind="ExternalInput")
    ctabs = [nc.dram_tensor(f"ctab{c}", [CROWS, ROWB], U8, kind="Internal")
             for c in range(NCH)]
    y = nc.dram_tensor("y", [1, SPC], F32, kind="ExternalOutput")

    with TileContext(nc) as tc:
        with (
            tc.tile_pool(name="const", bufs=1) as cpool,
            tc.tile_pool(name="acc", bufs=1) as apool,
            tc.tile_pool(name="stg", bufs=2) as spool,
            tc.tile_pool(name="sel", bufs=8) as lpool,
            tc.tile_pool(name="sq", bufs=3) as qpool,
            tc.tile_pool(name="psum", bufs=2, space="PSUM") as ppool,
            tc.tile_pool(name="psuml", bufs=1, space="PSUM") as plpool,
        ):
            v_sb = cpool.tile([128, TS], BF16)
            nc.sync.dma_start(out=v_sb[:], in_=vmat.ap())
            lam_sb = cpool.tile([TS, 1], F32)
            nc.sync.dma_start(out=lam_sb[:], in_=lam.ap())
            ones_sb = cpool.tile([128, 1], F32)
            nc.sync.dma_start(out=ones_sb[:], in_=onesv.ap())
            lin_sb = cpool.tile([128, PAYB], BF16)
            nc.sync.dma_start(out=lin_sb[:], in_=linp.ap())
            bffn_sb = cpool.tile([1, 1], F32)
            nc.sync.dma_start(out=bffn_sb[:], in_=bffn.ap())

            # prefetch ALL index tiles up front (removes load latency from
            # the gather critical path)
            i1_sb = cpool.tile([128, NCH, M1TOT16], I16)
            nc.sync.dma_start(out=i1_sb[:, 0, :], in_=i1.ap()[0])
            nc.sync.dma_start(out=i1_sb[:, 1:, :],
                              in_=i1.ap()[1:].rearrange("c p n -> p c n"))
            i2_sb = cpool.tile([128, NCH, NSUB, SUB * 128 // 16], I16)
            nc.sync.dma_start(out=i2_sb[:],
                              in_=i2.ap().rearrange("c s p n -> p c s n"))

            acc_sq = apool.tile([TS, SPC], F32)
            acc_lin = apool.tile([TS, SPC], F32)

            def emit_stage1(c, stg, k):
                if True:
                    r0 = k * CRANK
                    _raw_gather(
                        nc.gpsimd, stg[:, r0:r0 + CRANK, :],
                        btab.ap()[k * BROWS:(k + 1) * BROWS, 0:PAYB],
                        i1_sb[:, c, CB[k] // 16:(CB[k] + M1) // 16],
                        M1, PAYB, queue_num=k)
                    # writeback split across both HWDGE engines: its ~69us
                    # single-engine transfer sat on the stage-2 critical path
                    eng = nc.sync if k < 2 else nc.scalar
                    eng.dma_start(
                        out=ctabs[c].ap()[CB[k]:CB[k] + M1, 0:PAYB]
                        .rearrange("(p r) e -> p r e", p=128),
                        in_=stg[:, r0:r0 + CRANK, :])

            def emit_stage2_gather(c, s):
                sel = lpool.tile([128, SUB, PAYB], U8, tag="sel",
                                 name=f"sel{c}_{s}")
                _raw_gather(nc.gpsimd, sel[:],
                            ctabs[c].ap()[:, 0:PAYB],
                            i2_sb[:, c, s, :], SUB * 128, PAYB,
                            queue_num=s)
                return sel

            def emit_compute(c, sels):
                for s in range(NSUB):
                    sel8 = sels[s][:].bitcast(FP8)
                    ng = SUPER // GRP
                    for t in range(SUB // SUPER):
                        p = ppool.tile([TS, ng, 512], F32,
                                       space="PSUM", tag="p")
                        for g in range(ng):
                            nc.tensor.matmul(
                                out=p[:, g, 0:GRP * PAYB],
                                lhsT=v_sb[:],
                                rhs=sel8[:, t * SUPER + g * GRP:
                                         t * SUPER + (g + 1) * GRP, :],
                                start=True, stop=True)
                        base = c * CHS + s * SUB + t * SUPER
                        pv = p[:, :, 0:GRP * PAYB].rearrange(
                            "p g (b e) -> p g b e", e=PAYB)
                        sq = qpool.tile([TS, ng, GRP, F], BF16, tag="sq")
                        nc.scalar.activation(
                            sq[:], pv[:, :, :, 0:F],
                            mybir.ActivationFunctionType.Square)
                        nc.vector.tensor_reduce(
                            out=acc_sq[:, base:base + SUPER].rearrange(
                                "p (g b) -> p g b", g=ng),
                            in_=sq[:],
                            axis=mybir.AxisListType.X, op=mybir.AluOpType.add)
                        lin = qpool.tile([TS, ng, GRP, PAYB], BF16, tag="lin")
                        nc.vector.tensor_tensor(
                            out=lin[:], in0=pv,
                            in1=lin_sb[0:TS]
                            .rearrange("p (a b e) -> p a b e", a=1, b=1)
                            .to_broadcast([TS, ng, GRP, PAYB]),
                            op=mybir.AluOpType.mult)
                        nc.vector.tensor_reduce(
                            out=acc_lin[:, base:base + SUPER].rearrange(
                                "p (g b) -> p g b", g=ng),
                            in_=lin[:],
                            axis=mybir.AxisListType.X, op=mybir.AluOpType.add)

            # software pipeline: keep 4 queues generating continuously.
            # bracket -1: stage1(0); bracket c: stage1(c+1) then stage2(c).
            def new_stg(c):
                return spool.tile([128, NBANK * CRANK, PAYB], U8,
                                  tag="stg", name=f"stg{c}")

            stg = new_stg(0)
            for k in range(NBANK):
                emit_stage1(0, stg, k)
            pending = None  # (chunk, sels)
            for c in range(NCH):
                sels = []
                nstg = new_stg(c + 1) if c + 1 < NCH else None
                for k in range(NBANK):
                    if nstg is not None:
                        emit_stage1(c + 1, nstg, k)
                    sels.append(emit_stage2_gather(c, k))
                if pending is not None:
                    emit_compute(*pending)
                pending = (c, sels)
            emit_compute(*pending)

            pl = plpool.tile([1, SPC], F32, space="PSUM")
            for h in range((SPC + 511) // 512):
                sl = slice(h * 512, min((h + 1) * 512, SPC))
                nc.tensor.matmul(out=pl[:, sl], lhsT=lam_sb[:],
                                 rhs=acc_sq[:, sl], start=True, stop=False)
                nc.tensor.matmul(out=pl[:, sl], lhsT=ones_sb[0:TS],
                                 rhs=acc_lin[:, sl], start=False, stop=True)
            y_sb = cpool.tile([1, SPC], F32)
            nc.scalar.activation(y_sb[:], pl[:],
                                 mybir.ActivationFunctionType.Sigmoid,
                                 bias=bffn_sb[:, :])
            nc.sync.dma_start(out=y.ap(), in_=y_sb[:])

    nc.compile()
    return nc


def _wrap16(flat):
    """[N] int16 list -> [128, N//16] wrapped+replicated index tile."""
    n = flat.shape[0]
    w = flat.reshape(n // 16, 16).T
    return np.tile(w, (8, 1)).astype(np.int16)


def _host_prep(x, emb, w_deep, b_deep, w_ffn, b_ffn):
    x = np.asarray(x)
    emb = np.asarray(emb, dtype=np.float32)
    w_deep = np.asarray(w_deep, dtype=np.float32)
    b_deep = np.asarray(b_deep, dtype=np.float32)
    w_ffn = np.asarray(w_ffn, dtype=np.float32).reshape(-1)
    b_ffn = np.asarray(b_ffn, dtype=np.float32).reshape(-1)

    n_deep = TS * K
    n_fm = TS * (TS - 1) // 2
    w1 = w_ffn[:n_deep].reshape(TS, K)
    w2 = w_ffn[n_deep:n_deep + n_fm].astype(np.float64)
    w3 = w_ffn[n_deep + n_fm:].reshape(TS, F)

    iu, ju = np.triu_indices(TS, k=1)
    A = np.zeros((TS, TS), dtype=np.float64)
    A[iu, ju] = w2 / 2
    A = A + A.T
    lam, V = np.linalg.eigh(A)

    fp8_np = mybir.dt.np(FP8)
    bf16_np = mybir.dt.np(BF16)

    emb8 = (emb * EMB_SCALE).astype(fp8_np)                        # [V, 64]
    hfeat = (np.maximum(emb.astype(np.float64) @ w_deep + b_deep, 0.0)
             * H_SCALE).astype(fp8_np)                             # [V, 32]
    btab = np.zeros((NBANK * BROWS, ROWB), dtype=np.uint8)
    for k in range(NBANK):
        lo = k * WINW
        n = min(WINW, VOCAB - lo)
        if n <= 0:
            break
        rows = slice(k * BROWS, k * BROWS + n)
        btab[rows, 0:F] = emb8[lo:lo + n].view(np.uint8)
        btab[rows, F:PAYB] = hfeat[lo:lo + n].view(np.uint8)

    vz = np.zeros((128, TS), dtype=bf16_np)
    vz[:TS, :] = V.astype(bf16_np)
    lam_dev = (lam / (EMB_SCALE * EMB_SCALE)).astype(np.float32).reshape(TS, 1)
    onesz = np.zeros((128, 1), dtype=np.float32)
    onesz[:TS] = 1.0
    w3t = (V.T @ w3) / EMB_SCALE                                   # [TS, 64]
    w1t = (V.T @ w1) / H_SCALE                                     # [TS, 32]
    linp = np.zeros((128, PAYB), dtype=bf16_np)
    linp[:TS, 0:F] = w3t.astype(bf16_np)
    linp[:TS, F:PAYB] = w1t.astype(bf16_np)

    shared = {
        "btab": btab, "vmat": vz, "lam": lam_dev, "onesv": onesz,
        "linp": linp, "bffn": b_ffn.reshape(1, 1).astype(np.float32),
    }

    xi = x.astype(np.int64)
    in_maps = []
    for core in range(NCORES):
        xs = xi[core * SPC:(core + 1) * SPC]                       # [SPC, TS]
        i1 = np.zeros((NCH, 128, M1TOT16), dtype=np.int16)
        i2 = np.zeros((NCH, NSUB, 128, SUB * 128 // 16), dtype=np.int16)
        for c in range(NCH):
            xc = xs[c * CHS:(c + 1) * CHS]                         # [CHS, TS]
            bank = xc // WINW
            local = xc - bank * WINW                               # [CHS, TS]
            cid = np.zeros((CHS, TS), dtype=np.int64)
            for k in range(NBANK):
                msk = bank == k
                uniq = np.unique(local[msk])
                lst = np.full(M1, WINW, dtype=np.int64)            # pad->zeros
                pos0 = 1 if k == 0 else 0                          # id 0=zeros
                assert len(uniq) <= M1 - pos0, len(uniq)
                lst[pos0:pos0 + len(uniq)] = uniq
                i1[c, :, CB[k] // 16:(CB[k] + M1) // 16] = _wrap16(
                    lst.astype(np.int16))
                # list position i lands at staging (p=i%128, r=i//128),
                # written back to compact row CB[k] + p*CRANK + r
                pos = pos0 + np.arange(len(uniq))
                lut = np.zeros(WINW + 1, dtype=np.int64)
                lut[uniq] = CB[k] + (pos % 128) * CRANK + pos // 128
                cid[msk] = lut[local[msk]]
            slot = np.zeros((CHS, 128), dtype=np.int64)            # pads -> 0
            slot[:, :TS] = cid
            for s in range(NSUB):
                i2[c, s] = _wrap16(
                    slot[s * SUB:(s + 1) * SUB].reshape(-1).astype(np.int16))
        in_maps.append({"i1": i1, "i2": i2, **shared})
    return in_maps


def kernel(x, emb, w_deep, b_deep, w_ffn, b_ffn):
    if "nc" not in _cached:
        _cached["nc"] = build_nc()
    nc = _cached["nc"]
    in_maps = _host_prep(x, emb, w_deep, b_deep, w_ffn, b_ffn)
    trace = os.environ.get("KERNEL_TRACE", "") == "1"
    res = run_bass_kernel_spmd(nc, in_maps, core_ids=list(range(NCORES)),
                               trace=trace)
    if trace and res.exec_time_ns is not None:
        print(f"HW exec time: {res.exec_time_ns} ns")
        print(f"mean exec time: {res.mean_exec_time_ns} ns")
        if res.instructions_and_trace:
            print(f"trace: {res.instructions_and_trace[1]}")
    out = np.concatenate([res.results[c]["y"].reshape(SPC)
                          for c in range(NCORES)])
    return out.reshape(BS, 1).astype(np.float32)
